# revision 1
# baseline (speedup 1.0000x reference)
"""CRF mean-field (nn_CRF) Trainium2 kernel, SPMD over 8 NeuronCores.

Math: 5 iterations of
    p   = softmax(q, axis=classes)
    out_f = p @ K_f           for two Gaussian kernels K_f (spatial, bilateral)
    q   = unaries - compat @ (sw @ out_sp + bw @ out_bl)

Sharding: points N=8192 split column-wise across 8 cores (1024 each). Each
core owns a [8192, 1024] slab of the two N x N Gaussian kernels. The slab is
constant across iterations, so it is built ONCE (iteration 1) and reused:

  - build: PE computes the partial exponent G'' = f_i . f_j - 0.5|f_i|^2 via an
    augmented feature matmul (lhsT rows = [f ; -0.5|f|^2], rhs rows = [f ; 1]),
    ScalarE exponentiates straight out of PSUM into bf16 tiles. The remaining
    exp(-0.5|f_j|^2) factor is an exact fp32 per-column post-scale (so bf16
    rounding only enters through terms that average out over the contraction).
  - reuse: most tiles stay RESIDENT in SBUF across iterations; a few are
    cached in HBM (packed so DMA runs are 4-8KB per partition) and streamed
    back; the rest are rebuilt each iteration to balance ScalarE vs DMA.

All slab matmuls are bf16: TRN2's PE clock-gate (HAM) only registers
bf16-path activity (fp32/fp32r streams throttle to 1.2 GHz), and bf16 is
1 cyc/row. The two filters' p @ K matmuls target different PSUM column groups
(tile_position=(0,32)) so they run concurrently on the array.

Per iteration the cores exchange their local class distribution p (20KB bf16)
via AllGather; the iteration-1 softmax is computed on the host.
"""

import numpy as np
import ml_dtypes

C = 10          # classes
N = 8192        # points
S = 3           # spatial dims
R = 8           # cores
NL = N // R     # local points per core
KCH = N // 128  # 64 i-chunks
KP = KCH // 2   # 32 i-chunk pairs
JCH = NL // 128  # 8 local j-chunks
NITER = 5
THETA_GAMMA = 8.0

# per-pair storage kind: resident in SBUF / rebuilt each iter / HBM-streamed
# (counts chosen to balance SBUF capacity, ScalarE exp rate, and HBM BW)
_N_RES, _N_REB, _N_STR = 13, 6, 13
_RES, _REB, _STR = [], [], []
_deficit = {"res": 0.0, "reb": 0.0, "str": 0.0}
_share = {"res": _N_RES / KP, "reb": _N_REB / KP, "str": _N_STR / KP}
_lists = {"res": _RES, "reb": _REB, "str": _STR}
_caps = {"res": _N_RES, "reb": _N_REB, "str": _N_STR}
for _kp in range(KP):
    for _kind in _deficit:
        _deficit[_kind] += _share[_kind]
    _pick = max((k for k in _deficit if len(_lists[k]) < _caps[k]),
                key=lambda k: _deficit[k])
    _deficit[_pick] -= 1.0
    _lists[_pick].append(_kp)
PAIR_KIND = {kp: ("reb" if kp in _REB else "res" if kp in _RES else "str")
             for kp in range(KP)}
RES_IDX = {kp: i for i, kp in enumerate(_RES)}
STR_IDX = {kp: i for i, kp in enumerate(_STR)}

_CACHE = {}


def _build_program():
    import concourse.mybir as mybir
    import concourse.tile as tile
    from concourse import bacc
    from concourse.bass import ts, ds

    f32 = mybir.dt.float32
    bf16 = mybir.dt.bfloat16
    nc = bacc.Bacc("TRN2", target_bir_lowering=False, debug=False, num_devices=R)

    # ---- I/O ----
    fstack_full = nc.dram_tensor("fstack_full", [39, N], bf16, kind="ExternalInput")
    fstack_loc = nc.dram_tensor("fstack_loc", [39, NL], bf16, kind="ExternalInput")
    p_init = nc.dram_tensor("p_init", [N, C], bf16, kind="ExternalInput")
    unT_loc = nc.dram_tensor("unT_loc", [NL, C], f32, kind="ExternalInput")
    amT_sp = nc.dram_tensor("amT_sp", [C, C], f32, kind="ExternalInput")
    amT_bl = nc.dram_tensor("amT_bl", [C, C], f32, kind="ExternalInput")
    arep_sp = nc.dram_tensor("arep_sp", [C, NL], f32, kind="ExternalInput")
    arep_bl = nc.dram_tensor("arep_bl", [C, NL], f32, kind="ExternalInput")
    qT_out = nc.dram_tensor("qT_out", [NL, C], f32, kind="ExternalOutput")

    EXP = mybir.ActivationFunctionType.Exp

    with tile.TileContext(nc) as tc:
        with (
            tc.tile_pool(name="const", bufs=1) as const,
            tc.tile_pool(name="state", bufs=1) as state,
            tc.tile_pool(name="epool2", bufs=3) as epool2,
            tc.tile_pool(name="epool4", bufs=3) as epool4,
            tc.tile_pool(name="opool", bufs=2) as opool,
            tc.tile_pool(name="qpool", bufs=2) as qpool,
            tc.tile_pool(name="psG", bufs=2, space="PSUM") as psG,
            tc.tile_pool(name="psO", bufs=2, space="PSUM") as psO,
            tc.tile_pool(name="dram", bufs=2, space="DRAM") as dram,
            tc.tile_pool(name="cache", bufs=1, space="DRAM") as cache,
        ):
            # ---- load constants (feature stacks pre-cast to bf16 on host) ----
            ff_sb = const.tile([39, N], bf16, name="ff_sb")
            fl_sb = const.tile([39, NL], bf16, name="fl_sb")
            nc.sync.dma_start(ff_sb[:], fstack_full[:])
            nc.sync.dma_start(fl_sb[:], fstack_loc[:])
            amT_sp_sb = const.tile([C, C], f32, name="amT_sp_sb")
            amT_bl_sb = const.tile([C, C], f32, name="amT_bl_sb")
            arep_sp_sb = const.tile([C, NL], f32, name="arep_sp_sb")
            arep_bl_sb = const.tile([C, NL], f32, name="arep_bl_sb")
            un_sb = const.tile([128, JCH, C], f32, name="un_sb")
            nc.sync.dma_start(amT_sp_sb[:], amT_sp[:])
            nc.sync.dma_start(amT_bl_sb[:], amT_bl[:])
            nc.sync.dma_start(arep_sp_sb[:], arep_sp[:])
            nc.sync.dma_start(arep_bl_sb[:], arep_bl[:])
            nc.sync.dma_start(
                un_sb[:], unT_loc[:].rearrange("(j p) c -> p j c", p=128)
            )

            # SBUF-resident slab tiles and the HBM cache for streamed pairs
            eres = const.tile([128, len(_RES), 4096], bf16, name="eres")
            ecache = cache.tile([max(len(_STR), 1), 128, 4096], bf16, name="ecache")

            # full class distribution (bf16), rebuilt from the gather each iter
            p_sb = state.tile([128, KCH, C], bf16, name="p_sb")
            nc.sync.dma_start(p_sb[:], p_init[:].rearrange("(k p) c -> p k c", p=128))

            # local softmax scratch
            mx_sb = state.tile([128, JCH], f32, name="mx_sb")
            sm_sb = state.tile([128, JCH], f32, name="sm_sb")
            rs_sb = state.tile([128, JCH], f32, name="rs_sb")
            el_sb = state.tile([128, JCH, C], f32, name="el_sb")

            def rebuild_tile(k, h, dst_ap):
                """PE+ACT: build the bf16 [128, 1024] (sp|bl) tile into dst_ap."""
                jsl = ds(h * 512, 512)
                gt = psG.tile([128, 1024], f32, name="gt", tag="gt")
                nc.tensor.matmul(
                    gt[:, 0:512],
                    ff_sb[0:4, ts(k, 128)],
                    fl_sb[0:4, jsl],
                    start=True, stop=True,
                )
                nc.tensor.matmul(
                    gt[:, 512:1024],
                    ff_sb[32:39, ts(k, 128)],
                    fl_sb[32:39, jsl],
                    start=True, stop=True,
                )
                nc.scalar.activation(dst_ap, gt[:], EXP, bias=0.0, scale=1.0)

            def main_mms(po, k, h, sp_ap, bl_ap):
                jsl = ds(h * 512, 512)
                nc.tensor.matmul(
                    po[0:C, jsl], p_sb[:, k, :], sp_ap,
                    start=(k == 0), stop=(k == KCH - 1),
                )
                nc.tensor.matmul(
                    po[32:32 + C, jsl], p_sb[:, k, :], bl_ap,
                    tile_position=(0, 32),
                    start=(k == 0), stop=(k == KCH - 1),
                )

            for t in range(NITER):
                # ---- out[10, NL] accumulation over the slab ----
                po = psO.tile([32 + C, NL], f32, name="po")
                et4 = None
                et2 = None
                for k in range(KCH):
                    kp, k2 = k // 2, k % 2
                    kind = PAIR_KIND[kp]
                    for h in range(2):
                        if kind == "res":
                            off = k2 * 2048 + h * 1024
                            base = eres[:, RES_IDX[kp], :]
                            if t == 0:
                                rebuild_tile(k, h, base[:, off:off + 1024])
                            sp_ap = base[:, off:off + 512]
                            bl_ap = base[:, off + 512:off + 1024]
                        elif kind == "reb" or t == 0:
                            if h == 0:
                                et2 = epool2.tile([128, 2048], bf16, name="et2")
                            rebuild_tile(k, h, et2[:, h * 1024:(h + 1) * 1024])
                            sp_ap = et2[:, h * 1024:h * 1024 + 512]
                            bl_ap = et2[:, h * 1024 + 512:(h + 1) * 1024]
                            if kind == "str" and h == 1:
                                eng = nc.sync if STR_IDX[kp] % 2 == 0 else nc.gpsimd
                                eng.dma_start(
                                    ecache[STR_IDX[kp], :, k2 * 2048:(k2 + 1) * 2048],
                                    et2[:],
                                )
                        else:  # streamed, t >= 1
                            if k2 == 0 and h == 0:
                                et4 = epool4.tile([128, 4096], bf16, name="et4")
                                nc.sync.dma_start(
                                    et4[:, 0:2048], ecache[STR_IDX[kp], :, 0:2048]
                                )
                                nc.gpsimd.dma_start(
                                    et4[:, 2048:4096],
                                    ecache[STR_IDX[kp], :, 2048:4096],
                                )
                            off = k2 * 2048 + h * 1024
                            sp_ap = et4[:, off:off + 512]
                            bl_ap = et4[:, off + 512:off + 1024]
                        main_mms(po, k, h, sp_ap, bl_ap)

                # ---- q_loc.T = unT_loc + (out*a).T @ amT per filter ----
                ot_s = opool.tile([C, NL], f32, name="ot_s")
                ot_b = opool.tile([C, NL], f32, name="ot_b")
                nc.vector.tensor_mul(ot_s[:], po[0:C, :], arep_sp_sb[:])
                nc.vector.tensor_mul(ot_b[:], po[32:32 + C, :], arep_bl_sb[:])
                qa = psG.tile([128, JCH, C], f32, name="qa", tag="gt")
                for j in range(JCH):
                    nc.tensor.matmul(
                        qa[:, j, :], ot_s[:, ts(j, 128)], amT_sp_sb[:],
                        start=True, stop=False,
                    )
                    nc.tensor.matmul(
                        qa[:, j, :], ot_b[:, ts(j, 128)], amT_bl_sb[:],
                        start=False, stop=True,
                    )
                ql = qpool.tile([128, JCH, C], f32, name="ql")
                nc.vector.tensor_add(ql[:], qa[:], un_sb[:])

                if t < NITER - 1:
                    # ---- local softmax -> p shard (bf16) -> AllGather ----
                    nc.vector.reduce_max(mx_sb[:], ql[:], axis=mybir.AxisListType.X)
                    mx_b = mx_sb[:].unsqueeze(2).broadcast_to((128, JCH, C))
                    nc.vector.tensor_sub(el_sb[:], ql[:], mx_b)
                    nc.scalar.activation(el_sb[:], el_sb[:], EXP, bias=0.0, scale=1.0)
                    nc.vector.reduce_sum(sm_sb[:], el_sb[:], axis=mybir.AxisListType.X)
                    nc.vector.reciprocal(rs_sb[:], sm_sb[:])
                    rs_b = rs_sb[:].unsqueeze(2).broadcast_to((128, JCH, C))
                    pl = qpool.tile([128, JCH, C], bf16, name="pl")
                    nc.vector.tensor_mul(pl[:], el_sb[:], rs_b)

                    bi = dram.tile([NL, C], bf16, name="bi")
                    bo = dram.tile([N, C], bf16, addr_space="Shared", name="bo")
                    nc.sync.dma_start(
                        bi[:].rearrange("(j p) c -> p j c", p=128), pl[:]
                    )
                    nc.gpsimd.collective_compute(
                        "AllGather",
                        mybir.AluOpType.bypass,
                        replica_groups=[list(range(R))],
                        ins=[bi[:].opt()],
                        outs=[bo[:].opt()],
                    )
                    nc.sync.dma_start(
                        p_sb[:], bo[:].rearrange("(k p) c -> p k c", p=128)
                    )
                else:
                    nc.sync.dma_start(
                        qT_out[:].rearrange("(j p) c -> p j c", p=128), ql[:]
                    )

    nc.compile()
    return nc


def _get_program():
    if "nc" not in _CACHE:
        _CACHE["nc"] = _build_program()
    return _CACHE["nc"]


def _host_prep(unaries, feat, sw, bw, compat):
    f_sp = feat[:S] / THETA_GAMMA
    f_bl = feat
    f2_sp = np.sum(f_sp * f_sp, axis=0)
    f2_bl = np.sum(f_bl * f_bl, axis=0)

    fstack_full = np.zeros((39, N), dtype=np.float32)  # cast to bf16 at the end
    fstack_full[0:S] = f_sp
    fstack_full[S] = -0.5 * f2_sp
    fstack_full[32:38] = f_bl
    fstack_full[38] = -0.5 * f2_bl

    fstack_loc_full = np.zeros((39, N), dtype=np.float32)
    fstack_loc_full[0:S] = f_sp
    fstack_loc_full[S] = 1.0
    fstack_loc_full[32:38] = f_bl
    fstack_loc_full[38] = 1.0

    a_sp = np.exp(-0.5 * f2_sp).astype(np.float32)
    a_bl = np.exp(-0.5 * f2_bl).astype(np.float32)
    arep_sp = np.broadcast_to(a_sp[None, :], (C, N)).copy()
    arep_bl = np.broadcast_to(a_bl[None, :], (C, N)).copy()

    amT_sp = np.ascontiguousarray((-(compat @ sw)).T).astype(np.float32)
    amT_bl = np.ascontiguousarray((-(compat @ bw)).T).astype(np.float32)

    qT_init = np.ascontiguousarray(unaries.T).astype(np.float32)
    # iteration-1 softmax on the host
    mx = unaries.max(axis=0, keepdims=True)
    e = np.exp(unaries - mx, dtype=np.float32)
    p0 = (e / e.sum(axis=0, keepdims=True)).astype(np.float32)
    p0T = np.ascontiguousarray(p0.T).astype(ml_dtypes.bfloat16)
    fstack_full = fstack_full.astype(ml_dtypes.bfloat16)
    fstack_loc_full = fstack_loc_full.astype(ml_dtypes.bfloat16)
    return fstack_full, fstack_loc_full, arep_sp, arep_bl, amT_sp, amT_bl, qT_init, p0T


def _make_in_maps(inputs):
    unaries = np.asarray(inputs["unaries"], dtype=np.float32)
    feat = np.asarray(inputs["feat"], dtype=np.float32)
    sw = np.asarray(inputs["spatial_weights"], dtype=np.float32)
    bw = np.asarray(inputs["bilateral_weights"], dtype=np.float32)
    compat = np.asarray(inputs["compatibility_matrix"], dtype=np.float32)

    fstack_full, fstack_loc_full, arep_sp, arep_bl, amT_sp, amT_bl, qT_init, p0T = (
        _host_prep(unaries, feat, sw, bw, compat)
    )
    in_maps = []
    for r in range(R):
        jsl = slice(r * NL, (r + 1) * NL)
        in_maps.append({
            "fstack_full": fstack_full,
            "fstack_loc": np.ascontiguousarray(fstack_loc_full[:, jsl]),
            "p_init": p0T,
            "unT_loc": np.ascontiguousarray(qT_init[jsl]),
            "amT_sp": amT_sp,
            "amT_bl": amT_bl,
            "arep_sp": np.ascontiguousarray(arep_sp[:, jsl]),
            "arep_bl": np.ascontiguousarray(arep_bl[:, jsl]),
        })
    return in_maps


def kernel(unaries, feat, spatial_weights, bilateral_weights, compatibility_matrix):
    from concourse.bass_utils import run_bass_kernel_spmd

    in_maps = _make_in_maps({
        "unaries": unaries,
        "feat": feat,
        "spatial_weights": spatial_weights,
        "bilateral_weights": bilateral_weights,
        "compatibility_matrix": compatibility_matrix,
    })
    nc = _get_program()
    res = run_bass_kernel_spmd(nc, in_maps, core_ids=list(range(R)))

    q = np.empty((C, N), dtype=np.float32)
    for r in range(R):
        q[:, r * NL:(r + 1) * NL] = res.results[r]["qT_out"].T
    return q

